# revision 25
# baseline (speedup 1.0000x reference)
"""Trainium2 Bass kernel for nn_DocREModel_Triangle (DocRE block-bilinear model).

Strategy (8 NeuronCores, single SPMD NEFF, fully pair-sharded — no collectives):
  Core c owns batch b=c//4 and entity rows i in [6*(c%4), 6*(c%4)+6)
  -> 144 (i,j) pairs = the contiguous global row range [144c, 144c+144).

  Phase 1 (transposed layout: feature dim on partitions, pairs on the free
  dim): host-gathered mention rows -> exp/sum/log entity embeddings,
  host-gathered attention rows -> e_att^T via selector matmuls (h-major so
  psum drains in 2-head copies), pairwise head-products (split DVE/Pool by
  pair-half) + l-normalization, context vectors rs^T, and the two tanh
  extractors hs^T/ts^T [768, 144] bf16 with the hpart/tpart terms folded
  in as zero-padded indicator-matmul accumulations.

  Phase 2 (Wc folded into Wp on the host: WpWc = Wp @ Wc [49152, 97],
  4.75x fewer MACs than feature+classifier): per contraction tile (k, sp)
  a full-128-row selector matmul broadcasts 2 hs rows over 64 t-slots
  (PSUM), the hs*ts bilinear product lands in SBUF bf16 (Act-copy + DVE
  4x-mul, or direct DVE mul from PSUM), and one matmul per tile
  accumulates logits^T [97, 144] into ping-pong PSUM banks.  The group
  loop is interleaved into the hs-extractor loop (one-Et lag) so phase 2
  starts as soon as the first hs tile exists, and runs PIPE=5 groups deep
  (extractor accumulators share the b1ps psum ring).  Bias + self-pair
  mask + final assembly happen on the host.

  All DRAM inputs are host-pretransposed to partition-major layouts so
  every input DMA is 128 long contiguous runs (cheap HWDGE generation);
  every latency-critical DMA rides the compute-free sync (SP) queue.
"""

import numpy as np
import ml_dtypes

bf16 = ml_dtypes.bfloat16

B, L, H, NH = 2, 512, 768, 12
NE, NM = 24, 4
E, BS, C = 768, 64, 97
K = E // BS                      # 12 blocks
NCORE = 8
IPC = NE // 4                    # 6 i-rows per core
PL = IPC * NE                    # 144 local pairs
NSP = BS // 2                    # 32 s-pair tiles per k
NKT = K * NSP                    # 384 contraction tiles [128, PL]
GRP = 2                          # s-pair tiles fused per product group
NG = NKT // GRP                  # 192 groups
GPK = NSP // GRP                 # 16 groups per k
PIPE = 5                         # sel-matmul groups in flight ahead of product


def _host_prep(inputs):
    """Build the 8 per-core input maps from the full inputs."""
    seq = np.asarray(inputs["sequence_output"], dtype=np.float32)
    att = np.asarray(inputs["attention"], dtype=np.float32)
    Wh = np.asarray(inputs["Wh"], dtype=np.float32)
    Wt = np.asarray(inputs["Wt"], dtype=np.float32)
    Wp = np.asarray(inputs["Wp"], dtype=np.float32)
    Wc = np.asarray(inputs["Wc"], dtype=np.float32)
    bh = np.asarray(inputs["bh"], dtype=np.float32)
    bt = np.asarray(inputs["bt"], dtype=np.float32)
    mpos = np.asarray(inputs["mention_pos"]).astype(np.int64)

    # fold the classifier into the projection (weight constant folding)
    wpwc = Wp @ Wc                                     # [E*BS, C] f32
    # row (k, s, t) -> partition (si, t), tile a=(k, sp), with s = 2*sp+si;
    # partition-major so the DMA moves 128 long contiguous runs
    wpwc_t = np.ascontiguousarray(
        wpwc.reshape(K, NSP, 2, BS, C).transpose(2, 3, 0, 1, 4)
        .reshape(128, NKT, C).astype(bf16))

    def pmaj(w):  # [768, 768] -> [128, 6, 768] partition-major
        return np.ascontiguousarray(
            w.reshape(6, 128, E).transpose(1, 0, 2).astype(bf16))

    wh1 = pmaj(Wh[:H])
    wh2 = pmaj(Wh[H:])
    wt1 = pmaj(Wt[:H])
    wt2 = pmaj(Wt[H:])
    bh_t = np.ascontiguousarray(bh.reshape(6, 128).T.astype(np.float32))
    bt_t = np.ascontiguousarray(bt.reshape(6, 128).T.astype(np.float32))

    in_maps = []
    for c in range(NCORE):
        b = c // 4
        i0 = (c % 4) * IPC
        ents = list(range(NE)) + list(range(i0, i0 + IPC))  # 24 j-side + 6 i-side

        # host-gathered mention rows: 4 m-blocks at 32-partition alignment
        mg = np.zeros((128, H), dtype=np.float32)
        for m in range(NM):
            for e_i, ent in enumerate(ents):
                mg[m * 32 + e_i] = seq[b, mpos[b, ent, m]]

        # host-gathered attention rows [128, NH, L]: rows = 4m x 30ents (+8 pad)
        ai = np.zeros((128, NH), dtype=np.int32)
        for h in range(NH):
            for m in range(NM):
                for e_i, ent in enumerate(ents):
                    ai[m * 30 + e_i, h] = h * L + mpos[b, ent, m]
        # lt-major so the device can consume l-tiles as they arrive
        att_gb = np.ascontiguousarray(
            att[b].reshape(NH * L, L)[ai.T].transpose(1, 0, 2)
            .reshape(128, NH, 4, 128).transpose(0, 2, 1, 3).astype(bf16))

        seq_pm = np.ascontiguousarray(
            seq[b].reshape(4, 128, H).transpose(1, 0, 2).astype(bf16))

        in_maps.append(
            {
                "seq_bf": seq_pm,
                "ment_g": mg,
                "att_gb": att_gb,
                "wh1": wh1,
                "wh2": wh2,
                "wt1": wt1,
                "wt2": wt2,
                "bh_t": bh_t,
                "bt_t": bt_t,
                "wpwc": wpwc_t,
            }
        )
    return in_maps


def _build_consts():
    # S_att [120, 30]: sums the 4 mention rows per entity during the
    # attention transpose-matmul (the /4 mean cancels in the normalization)
    S = np.zeros((120, 30), dtype=bf16)
    for m in range(NM):
        for e_i in range(30):
            S[m * 30 + e_i, e_i] = 1.0
    # S2 [128, 30]: same for the 32-aligned mention-exp rows
    S2 = np.zeros((128, 30), dtype=bf16)
    for m in range(NM):
        for e_i in range(30):
            S2[m * 32 + e_i, e_i] = 1.0
    ones_bf = np.ones((128, 1), dtype=bf16)
    ones_row = np.ones((1, 128), dtype=np.float32)
    # sel128 [128, 64, 128]: variant v=(k%2)*32+sp broadcasts hs-tile rows
    # (k%2)*64 + (2sp, 2sp+1) into partitions 0..63 / 64..127.  Full 128-row
    # stationary keeps every phase-2 matmul in the same PE tile config.
    sel128 = np.zeros((128, 64, 128), dtype=bf16)
    for kb2 in range(2):
        for sp in range(NSP):
            v = kb2 * NSP + sp
            sel128[kb2 * 64 + 2 * sp, v, 0:64] = 1.0
            sel128[kb2 * 64 + 2 * sp + 1, v, 64:128] = 1.0
    # dupsel [128, 2, 128]: variant kb copies ts rows kb*64..kb*64+63 into
    # BOTH 64-partition halves (PE-side replacement for a partition-move DMA)
    dupsel = np.zeros((128, 2, 128), dtype=bf16)
    for kb in range(2):
        for p in range(128):
            dupsel[kb * 64 + (p % 64), kb, p] = 1.0
    # pair indicators padded to 128 rows so the fold-in matmul keeps the
    # same (128,128) PE tile config as the rs-contraction matmuls
    jsel = np.zeros((128, PL), dtype=bf16)
    isel = np.zeros((128, PL), dtype=bf16)
    for p in range(PL):
        jsel[p % NE, p] = 1.0
        isel[p // NE, p] = 1.0
    return S, S2, ones_bf, ones_row, sel128, dupsel, jsel, isel


def build_bass():
    import concourse.bass as bass
    import concourse.mybir as mybir
    import concourse.tile as tile
    from concourse.bacc import Bacc

    f32 = mybir.dt.float32
    bft = mybir.dt.bfloat16
    AF = mybir.ActivationFunctionType
    ALU = mybir.AluOpType

    nc = Bacc("TRN2", num_devices=NCORE)

    # ---- I/O ----
    seq_bf = nc.dram_tensor("seq_bf", [128, 4, H], bft, kind="ExternalInput")
    ment_g = nc.dram_tensor("ment_g", [128, H], f32, kind="ExternalInput")
    att_gb = nc.dram_tensor("att_gb", [128, 4, NH, 128], bft, kind="ExternalInput")
    wh1 = nc.dram_tensor("wh1", [128, 6, E], bft, kind="ExternalInput")
    wh2 = nc.dram_tensor("wh2", [128, 6, E], bft, kind="ExternalInput")
    wt1 = nc.dram_tensor("wt1", [128, 6, E], bft, kind="ExternalInput")
    wt2 = nc.dram_tensor("wt2", [128, 6, E], bft, kind="ExternalInput")
    bh_t = nc.dram_tensor("bh_t", [128, 6], f32, kind="ExternalInput")
    bt_t = nc.dram_tensor("bt_t", [128, 6], f32, kind="ExternalInput")
    wpwc = nc.dram_tensor("wpwc", [128, NKT, C], bft, kind="ExternalInput")
    out_lgT = nc.dram_tensor("out_lgT", [C, PL], f32, kind="ExternalOutput")

    (S_np, S2_np, ones_np, onesrow_np, sel128_np, dupsel_np, jsel_np,
     isel_np) = _build_consts()
    S_dr = nc.inline_tensor(S_np, "sel_const")
    S2_dr = nc.inline_tensor(S2_np, "s2_const")
    ones_dr = nc.inline_tensor(ones_np, "ones_const")
    onesrow_dr = nc.inline_tensor(onesrow_np, "onesrow_const")
    sel128_dr = nc.inline_tensor(sel128_np, "sel128_const")
    dupsel_dr = nc.inline_tensor(dupsel_np, "dupsel_const")
    jsel_dr = nc.inline_tensor(jsel_np, "jsel_const")
    isel_dr = nc.inline_tensor(isel_np, "isel_const")

    with tile.TileContext(nc) as tc:
        with (
            tc.tile_pool(name="gpool", bufs=1) as gpool,
            tc.tile_pool(name="persist", bufs=1) as persist,
        ):
            # ---------- whole-kernel-lifetime weights / constants ----------
            wp_a = gpool.tile([128, NKT // 2, C], bft)
            wp_b = gpool.tile([128, NKT // 2, C], bft)
            ones_sb = gpool.tile([128, 1], bft)
            onesrow_sb = gpool.tile([1, 128], f32)
            sel128_sb = gpool.tile([128, 64, 128], bft)
            dupsel_sb = gpool.tile([128, 2, 128], bft)
            jsel_sb = gpool.tile([128, PL], bft)
            isel_sb = gpool.tile([128, PL], bft)

            hs_sb = persist.tile([128, 6, PL], bft)
            ts_sb = persist.tile([128, 6, PL], bft)
            ts_dup = persist.tile([128, K, PL], bft)
            lg_sb = persist.tile([C, PL], f32)
            rsT = persist.tile([128, 6, PL], bft)
            tp2 = persist.tile([128, 2, 384], bft)
            hp2 = persist.tile([128, 2, 384], bft)
            wt2_sb = persist.tile([128, 6, E], bft)
            wh2_sb = persist.tile([128, 6, E], bft)
            bh_sb = persist.tile([128, 6], f32)
            bt_sb = persist.tile([128, 6], f32)

            with (
                tc.tile_pool(name="p1", bufs=1) as p1,
                tc.tile_pool(name="ps1", bufs=2, space="PSUM") as ps1,
            ):
                # ---------- input DMAs: everything latency-critical on the
                # compute-free sync queue, in need order ----------
                mg_sb = p1.tile([128, H], f32)
                nc.sync.dma_start(out=mg_sb, in_=ment_g[:])
                S2_sb = p1.tile([128, 30], bft)
                nc.sync.dma_start(out=S2_sb, in_=S2_dr[:])
                S_sb = p1.tile([120, 30], bft)
                nc.sync.dma_start(out=S_sb, in_=S_dr[:])
                att_b = p1.tile([128, 4, NH, 128], bft)
                for lt in range(4):
                    nc.sync.dma_start(out=att_b[:, lt, :, :], in_=att_gb[:, lt, :, :])
                seq_sb = p1.tile([128, 4, H], bft)
                nc.sync.dma_start(out=seq_sb, in_=seq_bf[:])
                wh1_sb = p1.tile([128, 6, E], bft)
                nc.sync.dma_start(out=wh1_sb, in_=wh1[:])
                wt1_sb = p1.tile([128, 6, E], bft)
                nc.sync.dma_start(out=wt1_sb, in_=wt1[:])
                nc.sync.dma_start(out=wt2_sb, in_=wt2[:])
                nc.sync.dma_start(out=wh2_sb, in_=wh2[:])
                nc.sync.dma_start(out=wp_a, in_=wpwc[:, 0 : NKT // 2, :])
                nc.sync.dma_start(out=wp_b, in_=wpwc[:, NKT // 2 :, :])
                # small / later-needed constants on the gpsimd (SWDGE) queue
                nc.gpsimd.dma_start(out=ones_sb, in_=ones_dr[:])
                nc.gpsimd.dma_start(out=onesrow_sb, in_=onesrow_dr[:])
                nc.gpsimd.dma_start(out=bh_sb, in_=bh_t[:])
                nc.gpsimd.dma_start(out=bt_sb, in_=bt_t[:])
                nc.gpsimd.dma_start(out=jsel_sb, in_=jsel_dr[:])
                nc.gpsimd.dma_start(out=isel_sb, in_=isel_dr[:])
                nc.gpsimd.dma_start(out=sel128_sb, in_=sel128_dr[:])
                nc.gpsimd.dma_start(out=dupsel_sb, in_=dupsel_dr[:])

                # ---------- mention path: logsumexp entity embeddings ----------
                exp_g = p1.tile([128, H], bft)
                nc.scalar.activation(out=exp_g, in_=mg_sb[:], func=AF.Exp)

                # eeT[h, ent] = ln(sum_m exp(ment)) via selector matmul
                eeT = p1.tile([128, 6, 30], bft)
                for ht in range(6):
                    tr = ps1.tile([128, 30], f32, tag="sm1", bufs=1)
                    nc.tensor.matmul(tr, lhsT=exp_g[:, 128 * ht : 128 * (ht + 1)],
                                     rhs=S2_sb[:], start=True, stop=True)
                    nc.scalar.activation(out=eeT[:, ht, :], in_=tr, func=AF.Ln)

                # ---------- attention path + pair products, pipelined per
                # l-tile; eaT is h-major so psum drains in 2-head copies ----------
                eaT = p1.tile([128, 4, NH, 30], bft)
                ht_raw = p1.tile([128, 4, PL], bft)
                htn = p1.tile([128, 4, PL], bft)
                sum_ps = ps1.tile([1, PL], f32, tag="lsum", bufs=1)
                with nc.allow_low_precision("bf16 pair-product reduce; normalization is scale-invariant"):
                    for lt in range(4):
                        for h2 in range(NH // 2):
                            ep = ps1.tile([128, 2, 30], f32, tag="ea", bufs=3)
                            for hh in range(2):
                                h = 2 * h2 + hh
                                nc.tensor.matmul(
                                    ep[:, hh, :],
                                    lhsT=att_b[0:120, lt, h, :],
                                    rhs=S_sb[:], start=True, stop=True)
                            nc.scalar.copy(out=eaT[:, lt, 2 * h2 : 2 * h2 + 2, :], in_=ep)
                        # products: prod[l, i, j, h] (h packed for the reduce),
                        # j-halves split DVE / Pool
                        prod = p1.tile([128, IPC, NE, NH], bft, tag="prod", bufs=2)
                        ea_i = eaT[:, lt, :, 24:30].rearrange("p h i -> p i h")
                        in0 = ea_i.unsqueeze(2).broadcast_to([128, IPC, 12, NH])
                        in1a = (eaT[:, lt, :, 0:12].rearrange("p h j -> p j h")
                                .unsqueeze(1).broadcast_to([128, IPC, 12, NH]))
                        in1b = (eaT[:, lt, :, 12:24].rearrange("p h j -> p j h")
                                .unsqueeze(1).broadcast_to([128, IPC, 12, NH]))
                        nc.vector.tensor_mul(out=prod[:, :, 0:12, :], in0=in0, in1=in1a)
                        nc.gpsimd.tensor_mul(out=prod[:, :, 12:24, :], in0=in0, in1=in1b)
                        nc.vector.tensor_reduce(
                            out=ht_raw[:, lt, :],
                            in_=prod[:].rearrange("p a b h -> p (a b) h"),
                            axis=mybir.AxisListType.X, op=ALU.add)
                        nc.vector.tensor_scalar_max(
                            out=ht_raw[:, lt, :], in0=ht_raw[:, lt, :], scalar1=0.0)
                        nc.tensor.matmul(sum_ps, lhsT=ones_sb[:], rhs=ht_raw[:, lt, :],
                                         start=(lt == 0), stop=(lt == 3))

                    # tpart/hpart in transposed [ent, e] form (folded into the
                    # extractors later); emitted here so the PE does them while
                    # the DVE/Pool product chains run
                    nc.vector.memset(tp2[:], 0.0)
                    nc.vector.memset(hp2[:], 0.0)
                    for half in range(2):
                        tpp = ps1.tile([NE, 384], f32, tag="tp2", bufs=1)
                        for ht in range(6):
                            nc.tensor.matmul(
                                tpp, lhsT=eeT[:, ht, 0:24],
                                rhs=wt1_sb[:, ht, 384 * half : 384 * (half + 1)],
                                start=(ht == 0), stop=(ht == 5))
                        nc.scalar.copy(out=tp2[0:NE, half, :], in_=tpp)
                        hpp = ps1.tile([IPC, 384], f32, tag="tp2", bufs=1)
                        for ht in range(6):
                            nc.tensor.matmul(
                                hpp, lhsT=eeT[:, ht, 24:30],
                                rhs=wh1_sb[:, ht, 384 * half : 384 * (half + 1)],
                                start=(ht == 0), stop=(ht == 5))
                        nc.scalar.copy(out=hp2[0:IPC, half, :], in_=hpp)

                    # ---------- normalization ----------
                    denom = p1.tile([1, PL], f32)
                    nc.vector.tensor_scalar_add(out=denom, in0=sum_ps, scalar1=1e-10)
                    recip = p1.tile([1, PL], f32)
                    nc.vector.reciprocal(out=recip, in_=denom)
                    rep_ps = ps1.tile([128, PL], f32, tag="acc", bufs=2)
                    nc.tensor.matmul(rep_ps, lhsT=onesrow_sb[:], rhs=recip[:], start=True, stop=True)
                    recip_rep = p1.tile([128, PL], f32)
                    nc.vector.tensor_copy(out=recip_rep, in_=rep_ps)
                    for lt in range(4):
                        nc.vector.tensor_mul(out=htn[:, lt, :], in0=ht_raw[:, lt, :], in1=recip_rep)

                # ---------- rs^T = seq^T @ ht_n ----------
                for ht in range(6):
                    rp = ps1.tile([128, PL], f32, tag="acc", bufs=2)
                    for lt in range(4):
                        nc.tensor.matmul(rp, lhsT=seq_sb[:, lt, 128 * ht : 128 * (ht + 1)],
                                         rhs=htn[:, lt, :], start=(lt == 0), stop=(lt == 3))
                    nc.scalar.copy(out=rsT[:, ht, :], in_=rp)

            # ---------- extractors + phase 2 (interleaved, shared psum ring) ----------
            with (
                tc.tile_pool(name="p2", bufs=1) as p2,
                tc.tile_pool(name="ps2", bufs=1, space="PSUM") as ps2,
            ):
                lg_a = ps2.tile([C, PL], f32, tag="lga", bufs=1)
                lg_b = ps2.tile([C, PL], f32, tag="lgb", bufs=1)

                def extractor(Et, w_sb, p2t, puse_sb, bias_sb, dst):
                    epx = ps2.tile([128, GRP, PL], f32, tag="b1ps", bufs=PIPE + 1)
                    for ht in range(6):
                        nc.tensor.matmul(epx[:, 0, :],
                                         lhsT=w_sb[:, ht, 128 * Et : 128 * (Et + 1)],
                                         rhs=rsT[:, ht, :], start=(ht == 0), stop=False)
                    nc.tensor.matmul(
                        epx[:, 0, :],
                        lhsT=p2t[:, Et // 3, 128 * (Et % 3) : 128 * (Et % 3) + 128],
                        rhs=puse_sb[:], start=False, stop=True)
                    nc.scalar.activation(out=dst[:, Et, :], in_=epx[:, 0, :], func=AF.Tanh,
                                         bias=bias_sb[:, Et : Et + 1])

                for Et in range(6):
                    extractor(Et, wt2_sb, tp2, jsel_sb, bt_sb, ts_sb)
                    # duplicate this tile's two k-blocks into both halves of
                    # ts_dup via a PE selector matmul + Act copy (no DMA queue
                    # traffic, no partition-move descriptors)
                    for kk in range(2):
                        dp = ps2.tile([128, GRP, PL], f32, tag="b1ps", bufs=PIPE + 1)
                        nc.tensor.matmul(dp[:, 0, :], lhsT=dupsel_sb[:, kk, :],
                                         rhs=ts_sb[:, Et, :], start=True, stop=True)
                        nc.scalar.copy(out=ts_dup[:, 2 * Et + kk, :], in_=dp[:, 0, :])

                b1ps_t = {}

                def sel_group(G):
                    k, g = divmod(G, GPK)
                    b1ps = ps2.tile([128, GRP, PL], f32, tag="b1ps", bufs=PIPE + 1)
                    b1ps_t[G] = b1ps
                    for cc in range(GRP):
                        v = (k % 2) * NSP + GRP * g + cc
                        nc.tensor.matmul(
                            b1ps[:, cc, :], lhsT=sel128_sb[:, v, :],
                            rhs=hs_sb[:, k // 2, :], start=True, stop=True)

                def fin_group(G):
                    k, g = divmod(G, GPK)
                    b1ps = b1ps_t.pop(G)
                    tsk = ts_dup[:, k, :].unsqueeze(1).broadcast_to([128, GRP, PL])
                    bl = p2.tile([128, GRP, PL], bft, tag="bl", bufs=PIPE + 1)
                    if G % 3 == 0:
                        # direct DVE multiply from PSUM (1x mode)
                        nc.vector.tensor_mul(out=bl, in0=b1ps[:], in1=tsk)
                    else:
                        # Act copies PSUM->SBUF bf16, DVE multiplies at 4x
                        b1c = p2.tile([128, GRP, PL], bft, tag="b1c", bufs=4)
                        nc.scalar.copy(out=b1c, in_=b1ps[:])
                        nc.vector.tensor_mul(out=bl, in0=b1c, in1=tsk)
                    for cc in range(GRP):
                        kt = k * NSP + GRP * g + cc
                        wp_sb = wp_a if kt < NKT // 2 else wp_b
                        lg_ps = lg_a if kt % 2 == 0 else lg_b
                        nc.tensor.matmul(
                            lg_ps, lhsT=wp_sb[:, kt % (NKT // 2), :],
                            rhs=bl[:, cc, :],
                            start=(kt < 2), stop=(kt >= NKT - 2))

                with nc.allow_low_precision("bf16 bilinear products, matches fp32-psum baseline error"):
                    G = 0
                    for Et in range(6):
                        extractor(Et, wh2_sb, hp2, isel_sb, bh_sb, hs_sb)
                        if Et > 0:
                            for _ in range(2 * GPK):   # groups for Et-1's k-blocks
                                sel_group(G)
                                if G >= PIPE:
                                    fin_group(G - PIPE)
                                G += 1
                    for _ in range(2 * GPK):           # groups for Et=5
                        sel_group(G)
                        fin_group(G - PIPE)
                        G += 1
                    for Gp in range(NG - PIPE, NG):    # drain
                        fin_group(Gp)
                    nc.scalar.copy(out=lg_sb, in_=lg_a[:])
                    nc.vector.tensor_add(out=lg_sb, in0=lg_sb, in1=lg_b[:])
                nc.sync.dma_start(out=out_lgT[:], in_=lg_sb[:])

    if not nc.is_finalized():
        nc.finalize()
    return nc


_NC_CACHE = None


def kernel(**inputs):
    global _NC_CACHE
    from concourse.bass_utils import run_bass_kernel_spmd

    if _NC_CACHE is None:
        _NC_CACHE = build_bass()
    in_maps = _host_prep(inputs)
    res = run_bass_kernel_spmd(_NC_CACHE, in_maps, core_ids=list(range(NCORE)))
    kernel.last_results = res
    bc = np.asarray(inputs["bc"], dtype=np.float32)
    full = np.empty((B * NE * NE, C), dtype=np.float32)
    for c in range(NCORE):
        full[PL * c : PL * (c + 1)] = np.asarray(
            res.results[c]["out_lgT"], dtype=np.float32).T
    full += bc[None, :]
    for b in range(B):
        for i in range(NE):
            full[NE * NE * b + (NE + 1) * i] = 0.0  # self-pair mask
    return full
